# revision 22
# baseline (speedup 1.0000x reference)
"""DirGATConv on 8 Trainium2 NeuronCores (Bass/Tile), v2 direct-gather fp16.

Strategy (node/data parallel, no collectives):
  - Each core owns 6250 destination nodes, bin-packed into 51 blocks of <=128
    so each (block, direction, src-bank) needs at most 5 chunks of 128 edges.
  - x is passed as two fp16 row banks (512-B rows, int16 gather indices).
    Per (block, dir, bank) one TRANSPOSE-mode dma_gather fetches the source
    rows already transposed: xg[p, k, i] = x[src_i][k*128+p], ready to be used
    as PE weights (lhsT) for the per-chunk projection.
  - Per chunk (128 edges): project h = x_src @ W_d (fp16, 2 k-chunks) and
    es = x_src @ w_es_d riding the same loaded weights (ldweights=False).
    ed[dst] is fetched via a tiny matmul with the host-built one-hot mask
    transpose MT: ed_c = MT^T @ ed_blk.  p = exp(leakyrelu(es+ed)) is computed
    batched per (block, dir): DVE add + fused max(x, 0.2x), exp on Scalar.
  - Aggregation: one fp16 matmul per chunk with the host-built 0/1 mask M as
    stationary weights: agg[:, 0:256] += M^T @ (h * p_bcast),
    agg[:, 256:260] += M^T @ p (softmax denominators), second matmul reuses
    the loaded mask weights (ldweights=False).
  - Softmax normalization after aggregation (numerator and denominator are
    both linear in p); combine directions with alpha=0.5 and add bias.
"""

import numpy as np

import concourse.bacc as bacc
import concourse.mybir as mybir
import concourse.tile as tile
from concourse.bass_utils import run_bass_kernel_spmd
from concourse import library_config

# problem constants
N, E, DIN, H, C = 50000, 400000, 256, 4, 64
HC = H * C
ALPHA, SLOPE = 0.5, 0.2

# distribution constants
NCORES = 8
NPC = N // NCORES              # 6250 destinations per core
BANK0 = 25088                  # x-bank split (int16 gather indices)
BANK1 = N + 48 - BANK0         # 24960; x padded to 50048 rows
NBIN = 51                      # destination blocks per core
CB = 5                         # chunks per (block, src-bank)
CPB = 2 * CB                   # chunks per block
NLOC = NBIN * 128              # 6528 local slots (perm order)
F32 = mybir.dt.float32
F16 = mybir.dt.float16
I16 = mybir.dt.int16

# z-scale engine per chunk parity: 'v' = DVE, 'g' = GpSimd
Z_ENG = "v"


def build_kernel():
    nc = bacc.Bacc("TRN2", num_swdge_queues=4)

    xb0 = nc.dram_tensor("xb0", [BANK0, DIN], F16, kind="ExternalInput")
    xb1 = nc.dram_tensor("xb1", [BANK1, DIN], F16, kind="ExternalInput")
    xtl = nc.dram_tensor("xtl", [DIN, NLOC], F16, kind="ExternalInput")
    wh = nc.dram_tensor("wh", [2, 2, 128, HC], F16, kind="ExternalInput")
    wes = nc.dram_tensor("wes", [2, 2, 128, H], F16, kind="ExternalInput")
    wed = nc.dram_tensor("wed", [2, 128, 2 * H], F16, kind="ExternalInput")
    bias = nc.dram_tensor("bias", [128, HC], F32, kind="ExternalInput")
    gidx = nc.dram_tensor("gidx", [2, NBIN, 128, 80], I16, kind="ExternalInput")
    mks = nc.dram_tensor("mks", [2, NBIN, 128, CPB, 128], F16,
                         kind="ExternalInput")
    mkt = nc.dram_tensor("mkt", [2, NBIN, 128, CPB, 128], F16,
                         kind="ExternalInput")
    out = nc.dram_tensor("out", [NLOC, HC], F32, kind="ExternalOutput")

    with tile.TileContext(nc) as tc:
        with tc.tile_pool(name="const", bufs=1) as cp:
            nc.gpsimd.load_library(library_config.mlp)

            # weights
            wh_sb = [cp.tile([128, 2, HC], F16, tag=f"wh{d}", name=f"wh{d}")
                     for d in range(2)]
            wes_sb = [cp.tile([128, 2, H], F16, tag=f"wes{d}", name=f"wes{d}")
                      for d in range(2)]
            for d in range(2):
                for k in range(2):
                    nc.sync.dma_start(wh_sb[d][:, k, :], wh[d, k, :, :])
                    nc.sync.dma_start(wes_sb[d][:, k, :], wes[d, k, :, :])
            wed_sb = cp.tile([128, 2, 2 * H], F16)
            for k in range(2):
                nc.sync.dma_start(wed_sb[:, k, :], wed[k, :, :])
            bias_sb = cp.tile([128, HC], F32)
            nc.sync.dma_start(bias_sb[:], bias[:])

            # ---------------- Phase A-lite: ed for local (permuted) nodes ---
            ed_sb = cp.tile([128, NBIN, 2 * H], F16, name="ed_sb")
            with (
                tc.tile_pool(name="pA", bufs=3) as pa,
                tc.tile_pool(name="psA", bufs=2, space="PSUM") as psa,
            ):
                for t in range(NBIN):
                    xlt = pa.tile([128, 2, 128], F16, tag="xlt")
                    for k in range(2):
                        nc.sync.dma_start(
                            xlt[:, k, :],
                            xtl[k * 128:(k + 1) * 128, t * 128:(t + 1) * 128])
                    ped = psa.tile([128, 2 * H], F32, tag="ped")
                    for k in range(2):
                        nc.tensor.matmul(ped[:], xlt[:, k, :], wed_sb[:, k, :],
                                         start=(k == 0), stop=(k == 1))
                    nc.vector.tensor_copy(ed_sb[:, t, :], ped[:])

            # ---------------- Phase B ----------------
            with (
                tc.tile_pool(name="pBg", bufs=3) as pg,
                tc.tile_pool(name="pBm", bufs=3) as pm,
                tc.tile_pool(name="pBs", bufs=4) as psb,
                tc.tile_pool(name="pBo", bufs=3) as po,
                tc.tile_pool(name="psH", bufs=4, space="PSUM") as psh,
                tc.tile_pool(name="psG", bufs=2, space="PSUM") as psg,
                tc.tile_pool(name="psL", bufs=2, space="PSUM") as psl,
            ):
                for b in range(NBIN):
                    stage = [None, None]
                    for d in range(2):
                        gi = pm.tile([128, 80], I16, tag="gi")
                        nc.sync.dma_start(gi[:], gidx[d, b, :, :])
                        xg = [pg.tile([128, 2, CB * 128], F16, tag=f"xg{bk}",
                                      name=f"xg{bk}") for bk in range(2)]
                        qq = (2 * b + d) * 2
                        nc.gpsimd.dma_gather(
                            xg[0][:], xb0[:], gi[:, 0:40], CB * 128, CB * 128,
                            DIN, transpose=True, queue_num=qq % 4)
                        nc.gpsimd.dma_gather(
                            xg[1][:], xb1[:], gi[:, 40:80], CB * 128, CB * 128,
                            DIN, transpose=True, queue_num=(qq + 1) % 4)
                        mks_t = pm.tile([128, CPB, 128], F16, tag="mks")
                        nc.sync.dma_start(mks_t[:], mks[d, b, :, :, :])
                        mkt_t = pm.tile([128, CPB, 128], F16, tag="mkt")
                        nc.sync.dma_start(mkt_t[:], mkt[d, b, :, :, :])

                        # PSUM banks (never touch a bank while an accumulation
                        # group is open in it):
                        #   num bank: [0:260] = M^T @ [z | p], one group/dir
                        #   lgp tile: es+ed per chunk, own bank via pool ring
                        num = psg.tile([128, 512], F32, tag="num")

                        for cc in range(CPB):
                            bk, c0 = divmod(cc, CB)
                            sl = slice(c0 * 128, (c0 + 1) * 128)
                            # lgp = ed (via mask transpose) + es, one region
                            lgp = psl.tile([128, H], F32, tag="lgp")
                            nc.tensor.matmul(
                                lgp[:], mkt_t[:, cc, :],
                                ed_sb[:, b, H * d:H * d + H],
                                start=True, stop=False)
                            if cc % 2 == 0:
                                hpair = psh.tile([128, 2, HC], F32, tag="hp")
                            hp = hpair[:, cc % 2, :]
                            for k in range(2):
                                nc.tensor.matmul(
                                    hp, xg[bk][:, k, sl], wh_sb[d][:, k, :],
                                    start=(k == 0), stop=(k == 1))
                                nc.tensor.matmul(
                                    lgp[:], xg[bk][:, k, sl],
                                    wes_sb[d][:, k, :],
                                    start=False, stop=(k == 1))
                            # p = exp(leakyrelu(es+ed)) on Scalar
                            lr = psb.tile([128, H], F16, tag="lr")
                            nc.scalar.activation(
                                lr[:], lgp[:],
                                mybir.ActivationFunctionType.Prelu,
                                alpha=SLOPE)
                            pt = psb.tile([128, H], F16, tag="pt")
                            nc.scalar.activation(
                                pt[:], lr[:],
                                mybir.ActivationFunctionType.Exp)
                            z = psb.tile([128, HC + H], F16, tag="z")
                            nc.gpsimd.tensor_copy(z[:, HC:HC + H], pt[:])
                            nc.vector.tensor_tensor(
                                z[:, 0:HC].rearrange("p (h c) -> p h c", h=H),
                                hp.rearrange("p (h c) -> p h c", h=H),
                                pt[:].unsqueeze(2).broadcast_to([128, H, C]),
                                mybir.AluOpType.mult)
                            nc.tensor.matmul(num[:, 0:HC + H],
                                             mks_t[:, cc, :], z[:],
                                             start=(cc == 0),
                                             stop=(cc == CPB - 1))
                        # normalize: stage = num / den  (den>0: self-loop)
                        den2 = po.tile([128, H], F32, tag="den2")
                        nc.vector.tensor_scalar(
                            out=den2[:], in0=num[:, HC:HC + H], scalar1=2.0,
                            scalar2=1e-12, op0=mybir.AluOpType.mult,
                            op1=mybir.AluOpType.max)
                        rec = po.tile([128, H], F32, tag="rec")
                        nc.vector.reciprocal(rec[:], den2[:])
                        stage[d] = po.tile([128, HC], F32, tag=f"st{d}",
                                           name=f"st{d}")
                        nc.vector.tensor_tensor(
                            stage[d][:].rearrange("p (h c) -> p h c", h=H),
                            num[:, 0:HC].rearrange("p (h c) -> p h c", h=H),
                            rec[:].unsqueeze(2).broadcast_to([128, H, C]),
                            mybir.AluOpType.mult)

                    ot = po.tile([128, HC], F32, tag="ot")
                    nc.gpsimd.tensor_tensor(ot[:], stage[0][:], stage[1][:],
                                            mybir.AluOpType.add)
                    ot2 = po.tile([128, HC], F32, tag="ot2")
                    nc.gpsimd.tensor_tensor(ot2[:], ot[:], bias_sb[:],
                                            mybir.AluOpType.add)
                    nc.sync.dma_start(out[b * 128:(b + 1) * 128, :], ot2[:])

    nc.compile()
    return nc


# ---------------------------------------------------------------- host side

def _wrap16(arr):
    """int idx array [n] -> dma_gather layout [128, n/16] int16 (replicated)."""
    n = len(arr)
    m = arr.reshape(n // 16, 16).astype(np.int16).T  # [16, n/16]
    return np.tile(m, (8, 1))


def prep_inputs(x, edge_index, W1, a_src1, a_dst1, b1, W2, a_src2, a_dst2, b2):
    x = np.asarray(x, np.float32)
    ei = np.asarray(edge_index)
    src, dst = ei[0].astype(np.int64), ei[1].astype(np.int64)
    loops = np.arange(N, dtype=np.int64)
    dirs = [
        (np.concatenate([src, loops]), np.concatenate([dst, loops])),
        (np.concatenate([dst, loops]), np.concatenate([src, loops])),
    ]

    x16 = x.astype(np.float16)
    xpad = np.zeros((BANK0 + BANK1, DIN), np.float16)
    xpad[:N] = x16
    xb0_h, xb1_h = xpad[:BANK0], xpad[BANK0:]

    Ws = [np.asarray(W1, np.float32), np.asarray(W2, np.float32)]
    asrc = [np.asarray(a_src1, np.float32), np.asarray(a_src2, np.float32)]
    adst = [np.asarray(a_dst1, np.float32), np.asarray(a_dst2, np.float32)]
    wh_h = np.zeros((2, 2, 128, HC), np.float16)
    wes_h = np.zeros((2, 2, 128, H), np.float16)
    wed_h = np.zeros((2, 128, 2 * H), np.float16)
    for d in range(2):
        Wd = Ws[d]
        w_es = np.stack([Wd[:, h * C:(h + 1) * C] @ asrc[d][h]
                         for h in range(H)], 1)       # [DIN, H]
        w_ed = np.stack([Wd[:, h * C:(h + 1) * C] @ adst[d][h]
                         for h in range(H)], 1)
        for k in range(2):
            wh_h[d, k] = Wd[k * 128:(k + 1) * 128, :].astype(np.float16)
            wes_h[d, k] = w_es[k * 128:(k + 1) * 128, :].astype(np.float16)
            wed_h[k, :, H * d:H * d + H] = \
                w_ed[k * 128:(k + 1) * 128, :].astype(np.float16)
    bias_h = np.broadcast_to(
        (0.5 * (np.asarray(b1) + np.asarray(b2))).astype(np.float32),
        (128, HC)).copy()

    # per-node degree by (dir, src-bank)
    deg = np.zeros((N, 4), np.int64)
    for j, (ss, dd) in enumerate(dirs):
        for bk in range(2):
            m = (ss >= BANK0) == (bk == 1)
            deg[:, 2 * j + bk] = np.bincount(dd[m], minlength=N)

    in_maps, perms = [], []
    for core in range(NCORES):
        lo = core * NPC
        nodes = np.arange(lo, lo + NPC)
        order = nodes[np.argsort(-deg[nodes].sum(1), kind="stable")]
        degs = deg[order]
        bins_load = np.zeros((NBIN, 4), np.int64)
        bins_cnt = np.zeros(NBIN, np.int64)
        node_blk = np.full(N, -1, np.int64)
        node_slot = np.full(N, -1, np.int64)
        for i_n in range(len(order)):
            dgl = degs[i_n]
            ok = (bins_cnt < 128) & ((bins_load + dgl) <= CB * 128).all(1)
            assert ok.any(), "bin packing failed; raise NBIN/CB"
            cand = np.where(ok)[0]
            nl = (bins_load[cand] + dgl).max(1) * 1000 + bins_cnt[cand]
            i = cand[np.argmin(nl)]
            node_blk[order[i_n]] = i
            node_slot[order[i_n]] = bins_cnt[i]
            bins_load[i] += dgl
            bins_cnt[i] += 1

        perm = np.full(NBIN * 128, -1, np.int64)
        perm[node_blk[nodes] * 128 + node_slot[nodes]] = nodes
        perms.append(perm)

        xtl_h = np.zeros((DIN, NLOC), np.float16)
        valid = perm >= 0
        xtl_h[:, valid] = x16[perm[valid]].T

        g_idx = np.zeros((2, NBIN, 128, 80), np.int16)
        mks_h = np.zeros((2, NBIN, CPB, 128, 128), np.float16)
        mkt_h = np.zeros((2, NBIN, CPB, 128, 128), np.float16)
        for d, (ss, dd) in enumerate(dirs):
            sel = (dd >= lo) & (dd < lo + NPC)
            es_, ed_ = ss[sel], dd[sel]
            blk = node_blk[ed_]
            bank = (es_ >= BANK0).astype(np.int64)
            eo = np.lexsort((bank, blk))
            es_, ed_, blk, bank = es_[eo], ed_[eo], blk[eo], bank[eo]
            seg = blk * 2 + bank
            segbnd = np.flatnonzero(np.diff(seg, prepend=-1))
            within = np.arange(len(seg)) - np.repeat(segbnd, np.diff(
                np.append(segbnd, len(seg))))
            assert (within < CB * 128).all()
            slot = within + np.where(bank == 0, 0, CB * 128)
            srcrel = np.where(bank == 0, es_, es_ - BANK0)
            s_idx = np.zeros((NBIN, CPB * 128), np.int64)
            s_idx[blk, slot] = srcrel
            # one-hot masks: edge at (chunk cc, lane e) -> dst slot
            cc_all = slot // 128
            lane = slot % 128
            dslot = node_slot[ed_]
            mks_h[d, blk, cc_all, lane, dslot] = 1.0
            mkt_h[d, blk, cc_all, dslot, lane] = 1.0
            for bb in range(NBIN):
                g_idx[d, bb, :, 0:40] = _wrap16(s_idx[bb, 0:CB * 128])
                g_idx[d, bb, :, 40:80] = _wrap16(s_idx[bb, CB * 128:])

        in_maps.append({
            "xb0": xb0_h, "xb1": xb1_h, "xtl": xtl_h,
            "wh": wh_h, "wes": wes_h, "wed": wed_h, "bias": bias_h,
            "gidx": g_idx,
            "mks": np.ascontiguousarray(mks_h.transpose(0, 1, 3, 2, 4)),
            "mkt": np.ascontiguousarray(mkt_h.transpose(0, 1, 3, 2, 4)),
        })
    return in_maps, perms


_NC_CACHE = {}


def kernel(**inputs):
    in_maps, perms = prep_inputs(**inputs)
    key = "k2"
    if key not in _NC_CACHE:
        _NC_CACHE[key] = build_kernel()
    nc = _NC_CACHE[key]
    res = run_bass_kernel_spmd(nc, in_maps, list(range(NCORES)))
    result = np.empty((N, HC), np.float32)
    for core in range(NCORES):
        o = res.results[core]["out"]
        p = perms[core]
        valid = p >= 0
        result[p[valid]] = o[valid]
    return result


# revision 23
# speedup vs baseline: 1.4059x; 1.4059x over previous
"""DirGATConv on 8 Trainium2 NeuronCores (Bass/Tile), v2 direct-gather fp16.

Strategy (node/data parallel, no collectives):
  - Each core owns 6250 destination nodes, bin-packed into 51 blocks of <=128
    so each (block, direction, src-bank) needs at most 5 chunks of 128 edges.
  - x is passed as two fp16 row banks (512-B rows, int16 gather indices).
    Per (block, dir, bank) one TRANSPOSE-mode dma_gather fetches the source
    rows already transposed: xg[p, k, i] = x[src_i][k*128+p], ready to be used
    as PE weights (lhsT) for the per-chunk projection.
  - Per chunk (128 edges): project h = x_src @ W_d (fp16, 2 k-chunks) and
    es = x_src @ w_es_d riding the same loaded weights (ldweights=False).
    ed[dst] is fetched via a tiny matmul with the host-built one-hot mask
    transpose MT: ed_c = MT^T @ ed_blk.  p = exp(leakyrelu(es+ed)) is computed
    batched per (block, dir): DVE add + fused max(x, 0.2x), exp on Scalar.
  - Aggregation: one fp16 matmul per chunk with the host-built 0/1 mask M as
    stationary weights: agg[:, 0:256] += M^T @ (h * p_bcast),
    agg[:, 256:260] += M^T @ p (softmax denominators), second matmul reuses
    the loaded mask weights (ldweights=False).
  - Softmax normalization after aggregation (numerator and denominator are
    both linear in p); combine directions with alpha=0.5 and add bias.
"""

import numpy as np

import concourse.bacc as bacc
import concourse.mybir as mybir
import concourse.tile as tile
from concourse.bass_utils import run_bass_kernel_spmd
from concourse import library_config

# problem constants
N, E, DIN, H, C = 50000, 400000, 256, 4, 64
HC = H * C
ALPHA, SLOPE = 0.5, 0.2

# distribution constants
NCORES = 8
NPC = N // NCORES              # 6250 destinations per core
BANK0 = 25088                  # x-bank split (int16 gather indices)
BANK1 = N + 48 - BANK0         # 24960; x padded to 50048 rows
NBIN = 51                      # destination blocks per core
CB = 5                         # chunks per (block, src-bank)
CPB = 2 * CB                   # chunks per block
NLOC = NBIN * 128              # 6528 local slots (perm order)
F32 = mybir.dt.float32
F16 = mybir.dt.float16
I16 = mybir.dt.int16

# z-scale engine per chunk parity: 'v' = DVE, 'g' = GpSimd
Z_ENG = "v"


def build_kernel():
    nc = bacc.Bacc("TRN2", num_swdge_queues=4)

    xb0 = nc.dram_tensor("xb0", [BANK0, DIN], F16, kind="ExternalInput")
    xb1 = nc.dram_tensor("xb1", [BANK1, DIN], F16, kind="ExternalInput")
    xtl = nc.dram_tensor("xtl", [DIN, NLOC], F16, kind="ExternalInput")
    wh = nc.dram_tensor("wh", [2, 2, 128, HC], F16, kind="ExternalInput")
    wes = nc.dram_tensor("wes", [2, 2, 128, H], F16, kind="ExternalInput")
    wed = nc.dram_tensor("wed", [2, 128, 2 * H], F16, kind="ExternalInput")
    bias = nc.dram_tensor("bias", [128, HC], F32, kind="ExternalInput")
    gidx = nc.dram_tensor("gidx", [2, NBIN, 128, 80], I16, kind="ExternalInput")
    mks = nc.dram_tensor("mks", [2, NBIN, 128, CPB, 128], F16,
                         kind="ExternalInput")
    mkt = nc.dram_tensor("mkt", [2, NBIN, 128, CPB, 128], F16,
                         kind="ExternalInput")
    out = nc.dram_tensor("out", [NLOC, HC], F32, kind="ExternalOutput")

    with tile.TileContext(nc) as tc:
        with tc.tile_pool(name="const", bufs=1) as cp:
            nc.gpsimd.load_library(library_config.mlp)

            # weights
            wh_sb = [cp.tile([128, 2, HC], F16, tag=f"wh{d}", name=f"wh{d}")
                     for d in range(2)]
            wes_sb = [cp.tile([128, 2, H], F16, tag=f"wes{d}", name=f"wes{d}")
                      for d in range(2)]
            for d in range(2):
                for k in range(2):
                    nc.sync.dma_start(wh_sb[d][:, k, :], wh[d, k, :, :])
                    nc.sync.dma_start(wes_sb[d][:, k, :], wes[d, k, :, :])
            wed_sb = cp.tile([128, 2, 2 * H], F16)
            for k in range(2):
                nc.sync.dma_start(wed_sb[:, k, :], wed[k, :, :])
            bias_sb = cp.tile([128, HC], F32)
            nc.sync.dma_start(bias_sb[:], bias[:])

            # ---------------- Phase A-lite: ed for local (permuted) nodes ---
            ed_sb = cp.tile([128, NBIN, 2 * H], F16, name="ed_sb")
            with (
                tc.tile_pool(name="pA", bufs=3) as pa,
                tc.tile_pool(name="psA", bufs=2, space="PSUM") as psa,
            ):
                for t in range(NBIN):
                    xlt = pa.tile([128, 2, 128], F16, tag="xlt")
                    for k in range(2):
                        nc.sync.dma_start(
                            xlt[:, k, :],
                            xtl[k * 128:(k + 1) * 128, t * 128:(t + 1) * 128])
                    ped = psa.tile([128, 2 * H], F32, tag="ped")
                    for k in range(2):
                        nc.tensor.matmul(ped[:], xlt[:, k, :], wed_sb[:, k, :],
                                         start=(k == 0), stop=(k == 1))
                    nc.vector.tensor_copy(ed_sb[:, t, :], ped[:])

            # ---------------- Phase B ----------------
            with (
                tc.tile_pool(name="pBg", bufs=3) as pg,
                tc.tile_pool(name="pBm", bufs=3) as pm,
                tc.tile_pool(name="pBs", bufs=4) as psb,
                tc.tile_pool(name="pBo", bufs=3) as po,
                tc.tile_pool(name="psH", bufs=4, space="PSUM") as psh,
                tc.tile_pool(name="psG", bufs=2, space="PSUM") as psg,
                tc.tile_pool(name="psL", bufs=2, space="PSUM") as psl,
            ):
                for b in range(NBIN):
                    stage = [None, None]
                    for d in range(2):
                        gi = pm.tile([128, 80], I16, tag="gi")
                        nc.sync.dma_start(gi[:], gidx[d, b, :, :])
                        xg = [pg.tile([128, 2, CB * 128], F16, tag=f"xg{bk}",
                                      name=f"xg{bk}") for bk in range(2)]
                        qq = (2 * b + d) * 2
                        nc.gpsimd.dma_gather(
                            xg[0][:], xb0[:], gi[:, 0:40], CB * 128, CB * 128,
                            DIN, transpose=True, queue_num=qq % 4)
                        nc.gpsimd.dma_gather(
                            xg[1][:], xb1[:], gi[:, 40:80], CB * 128, CB * 128,
                            DIN, transpose=True, queue_num=(qq + 1) % 4)
                        mks_t = pm.tile([128, CPB, 128], F16, tag="mks")
                        nc.sync.dma_start(mks_t[:], mks[d, b, :, :, :])
                        mkt_t = pm.tile([128, CPB, 128], F16, tag="mkt")
                        nc.sync.dma_start(mkt_t[:], mkt[d, b, :, :, :])

                        # PSUM banks (never touch a bank while an accumulation
                        # group is open in it):
                        #   num bank: [0:260] = M^T @ [z | p], one group/dir
                        #   lgp tile: es+ed per chunk, own bank via pool ring
                        num = psg.tile([128, 512], F32, tag="num")

                        for cc in range(CPB):
                            bk, c0 = divmod(cc, CB)
                            sl = slice(c0 * 128, (c0 + 1) * 128)
                            # lgp = ed (via mask transpose) + es, one region
                            lgp = psl.tile([128, H], F32, tag="lgp")
                            nc.tensor.matmul(
                                lgp[:], mkt_t[:, cc, :],
                                ed_sb[:, b, H * d:H * d + H],
                                start=True, stop=False)
                            if cc % 2 == 0:
                                hpair = psh.tile([128, 2, HC], F32, tag="hp")
                            hp = hpair[:, cc % 2, :]
                            for k in range(2):
                                nc.tensor.matmul(
                                    hp, xg[bk][:, k, sl], wh_sb[d][:, k, :],
                                    start=(k == 0), stop=(k == 1))
                                nc.tensor.matmul(
                                    lgp[:], xg[bk][:, k, sl],
                                    wes_sb[d][:, k, :],
                                    start=False, stop=(k == 1))
                            # p = exp(leakyrelu(es+ed)) on Scalar
                            lr = psb.tile([128, H], F16, tag="lr")
                            nc.scalar.activation(
                                lr[:], lgp[:],
                                mybir.ActivationFunctionType.Prelu,
                                alpha=SLOPE)
                            z = psb.tile([128, HC + H], F16, tag="z")
                            nc.scalar.activation(
                                z[:, HC:HC + H], lr[:],
                                mybir.ActivationFunctionType.Exp)
                            nc.vector.tensor_tensor(
                                z[:, 0:HC].rearrange("p (h c) -> p h c", h=H),
                                hp.rearrange("p (h c) -> p h c", h=H),
                                z[:, HC:HC + H].unsqueeze(2).broadcast_to(
                                    [128, H, C]),
                                mybir.AluOpType.mult)
                            nc.tensor.matmul(num[:, 0:HC + H],
                                             mks_t[:, cc, :], z[:],
                                             start=(cc == 0),
                                             stop=(cc == CPB - 1))
                        # normalize: stage = num / den  (den>0: self-loop)
                        den2 = po.tile([128, H], F32, tag="den2")
                        nc.vector.tensor_scalar(
                            out=den2[:], in0=num[:, HC:HC + H], scalar1=2.0,
                            scalar2=1e-12, op0=mybir.AluOpType.mult,
                            op1=mybir.AluOpType.max)
                        rec = po.tile([128, H], F32, tag="rec")
                        nc.vector.reciprocal(rec[:], den2[:])
                        stage[d] = po.tile([128, HC], F32, tag=f"st{d}",
                                           name=f"st{d}")
                        nc.vector.tensor_tensor(
                            stage[d][:].rearrange("p (h c) -> p h c", h=H),
                            num[:, 0:HC].rearrange("p (h c) -> p h c", h=H),
                            rec[:].unsqueeze(2).broadcast_to([128, H, C]),
                            mybir.AluOpType.mult)

                    ot = po.tile([128, HC], F32, tag="ot")
                    nc.gpsimd.tensor_tensor(ot[:], stage[0][:], stage[1][:],
                                            mybir.AluOpType.add)
                    ot2 = po.tile([128, HC], F32, tag="ot2")
                    nc.gpsimd.tensor_tensor(ot2[:], ot[:], bias_sb[:],
                                            mybir.AluOpType.add)
                    nc.sync.dma_start(out[b * 128:(b + 1) * 128, :], ot2[:])

    nc.compile()
    return nc


# ---------------------------------------------------------------- host side

def _wrap16(arr):
    """int idx array [n] -> dma_gather layout [128, n/16] int16 (replicated)."""
    n = len(arr)
    m = arr.reshape(n // 16, 16).astype(np.int16).T  # [16, n/16]
    return np.tile(m, (8, 1))


def prep_inputs(x, edge_index, W1, a_src1, a_dst1, b1, W2, a_src2, a_dst2, b2):
    x = np.asarray(x, np.float32)
    ei = np.asarray(edge_index)
    src, dst = ei[0].astype(np.int64), ei[1].astype(np.int64)
    loops = np.arange(N, dtype=np.int64)
    dirs = [
        (np.concatenate([src, loops]), np.concatenate([dst, loops])),
        (np.concatenate([dst, loops]), np.concatenate([src, loops])),
    ]

    x16 = x.astype(np.float16)
    xpad = np.zeros((BANK0 + BANK1, DIN), np.float16)
    xpad[:N] = x16
    xb0_h, xb1_h = xpad[:BANK0], xpad[BANK0:]

    Ws = [np.asarray(W1, np.float32), np.asarray(W2, np.float32)]
    asrc = [np.asarray(a_src1, np.float32), np.asarray(a_src2, np.float32)]
    adst = [np.asarray(a_dst1, np.float32), np.asarray(a_dst2, np.float32)]
    wh_h = np.zeros((2, 2, 128, HC), np.float16)
    wes_h = np.zeros((2, 2, 128, H), np.float16)
    wed_h = np.zeros((2, 128, 2 * H), np.float16)
    for d in range(2):
        Wd = Ws[d]
        w_es = np.stack([Wd[:, h * C:(h + 1) * C] @ asrc[d][h]
                         for h in range(H)], 1)       # [DIN, H]
        w_ed = np.stack([Wd[:, h * C:(h + 1) * C] @ adst[d][h]
                         for h in range(H)], 1)
        for k in range(2):
            wh_h[d, k] = Wd[k * 128:(k + 1) * 128, :].astype(np.float16)
            wes_h[d, k] = w_es[k * 128:(k + 1) * 128, :].astype(np.float16)
            wed_h[k, :, H * d:H * d + H] = \
                w_ed[k * 128:(k + 1) * 128, :].astype(np.float16)
    bias_h = np.broadcast_to(
        (0.5 * (np.asarray(b1) + np.asarray(b2))).astype(np.float32),
        (128, HC)).copy()

    # per-node degree by (dir, src-bank)
    deg = np.zeros((N, 4), np.int64)
    for j, (ss, dd) in enumerate(dirs):
        for bk in range(2):
            m = (ss >= BANK0) == (bk == 1)
            deg[:, 2 * j + bk] = np.bincount(dd[m], minlength=N)

    in_maps, perms = [], []
    for core in range(NCORES):
        lo = core * NPC
        nodes = np.arange(lo, lo + NPC)
        order = nodes[np.argsort(-deg[nodes].sum(1), kind="stable")]
        degs = deg[order]
        bins_load = np.zeros((NBIN, 4), np.int64)
        bins_cnt = np.zeros(NBIN, np.int64)
        node_blk = np.full(N, -1, np.int64)
        node_slot = np.full(N, -1, np.int64)
        for i_n in range(len(order)):
            dgl = degs[i_n]
            ok = (bins_cnt < 128) & ((bins_load + dgl) <= CB * 128).all(1)
            assert ok.any(), "bin packing failed; raise NBIN/CB"
            cand = np.where(ok)[0]
            nl = (bins_load[cand] + dgl).max(1) * 1000 + bins_cnt[cand]
            i = cand[np.argmin(nl)]
            node_blk[order[i_n]] = i
            node_slot[order[i_n]] = bins_cnt[i]
            bins_load[i] += dgl
            bins_cnt[i] += 1

        perm = np.full(NBIN * 128, -1, np.int64)
        perm[node_blk[nodes] * 128 + node_slot[nodes]] = nodes
        perms.append(perm)

        xtl_h = np.zeros((DIN, NLOC), np.float16)
        valid = perm >= 0
        xtl_h[:, valid] = x16[perm[valid]].T

        g_idx = np.zeros((2, NBIN, 128, 80), np.int16)
        mks_h = np.zeros((2, NBIN, CPB, 128, 128), np.float16)
        mkt_h = np.zeros((2, NBIN, CPB, 128, 128), np.float16)
        for d, (ss, dd) in enumerate(dirs):
            sel = (dd >= lo) & (dd < lo + NPC)
            es_, ed_ = ss[sel], dd[sel]
            blk = node_blk[ed_]
            bank = (es_ >= BANK0).astype(np.int64)
            eo = np.lexsort((bank, blk))
            es_, ed_, blk, bank = es_[eo], ed_[eo], blk[eo], bank[eo]
            seg = blk * 2 + bank
            segbnd = np.flatnonzero(np.diff(seg, prepend=-1))
            within = np.arange(len(seg)) - np.repeat(segbnd, np.diff(
                np.append(segbnd, len(seg))))
            assert (within < CB * 128).all()
            slot = within + np.where(bank == 0, 0, CB * 128)
            srcrel = np.where(bank == 0, es_, es_ - BANK0)
            s_idx = np.zeros((NBIN, CPB * 128), np.int64)
            s_idx[blk, slot] = srcrel
            # one-hot masks: edge at (chunk cc, lane e) -> dst slot
            cc_all = slot // 128
            lane = slot % 128
            dslot = node_slot[ed_]
            mks_h[d, blk, cc_all, lane, dslot] = 1.0
            mkt_h[d, blk, cc_all, dslot, lane] = 1.0
            for bb in range(NBIN):
                g_idx[d, bb, :, 0:40] = _wrap16(s_idx[bb, 0:CB * 128])
                g_idx[d, bb, :, 40:80] = _wrap16(s_idx[bb, CB * 128:])

        in_maps.append({
            "xb0": xb0_h, "xb1": xb1_h, "xtl": xtl_h,
            "wh": wh_h, "wes": wes_h, "wed": wed_h, "bias": bias_h,
            "gidx": g_idx,
            "mks": np.ascontiguousarray(mks_h.transpose(0, 1, 3, 2, 4)),
            "mkt": np.ascontiguousarray(mkt_h.transpose(0, 1, 3, 2, 4)),
        })
    return in_maps, perms


_NC_CACHE = {}


def kernel(**inputs):
    in_maps, perms = prep_inputs(**inputs)
    key = "k2"
    if key not in _NC_CACHE:
        _NC_CACHE[key] = build_kernel()
    nc = _NC_CACHE[key]
    res = run_bass_kernel_spmd(nc, in_maps, list(range(NCORES)))
    result = np.empty((N, HC), np.float32)
    for core in range(NCORES):
        o = res.results[core]["out"]
        p = perms[core]
        valid = p >= 0
        result[p[valid]] = o[valid]
    return result


# revision 25
# speedup vs baseline: 1.5307x; 1.0888x over previous
"""DirGATConv on 8 Trainium2 NeuronCores (Bass/Tile), v2 direct-gather fp16.

Strategy (node/data parallel, no collectives):
  - Each core owns 6250 destination nodes, bin-packed into 51 blocks of <=128
    so each (block, direction, src-bank) needs at most 5 chunks of 128 edges.
  - x is passed as two fp16 row banks (512-B rows, int16 gather indices).
    Per (block, dir, bank) one TRANSPOSE-mode dma_gather fetches the source
    rows already transposed: xg[p, k, i] = x[src_i][k*128+p], ready to be used
    as PE weights (lhsT) for the per-chunk projection.
  - Per chunk (128 edges): project h = x_src @ W_d (fp16, 2 k-chunks) and
    es = x_src @ w_es_d riding the same loaded weights (ldweights=False).
    ed[dst] is fetched via a tiny matmul with the host-built one-hot mask
    transpose MT: ed_c = MT^T @ ed_blk.  p = exp(leakyrelu(es+ed)) is computed
    batched per (block, dir): DVE add + fused max(x, 0.2x), exp on Scalar.
  - Aggregation: one fp16 matmul per chunk with the host-built 0/1 mask M as
    stationary weights: agg[:, 0:256] += M^T @ (h * p_bcast),
    agg[:, 256:260] += M^T @ p (softmax denominators), second matmul reuses
    the loaded mask weights (ldweights=False).
  - Softmax normalization after aggregation (numerator and denominator are
    both linear in p); combine directions with alpha=0.5 and add bias.
"""

import numpy as np

import concourse.bacc as bacc
import concourse.mybir as mybir
import concourse.tile as tile
from concourse.bass_utils import run_bass_kernel_spmd
from concourse import library_config

# problem constants
N, E, DIN, H, C = 50000, 400000, 256, 4, 64
HC = H * C
ALPHA, SLOPE = 0.5, 0.2

# distribution constants
NCORES = 8
NPC = N // NCORES              # 6250 destinations per core
BANK0 = 25088                  # x-bank split (int16 gather indices)
BANK1 = N + 48 - BANK0         # 24960; x padded to 50048 rows
NBIN = 51                      # destination blocks per core
CB = 5                         # chunks per (block, src-bank)
CPB = 2 * CB                   # chunks per block
NLOC = NBIN * 128              # 6528 local slots (perm order)
F32 = mybir.dt.float32
F16 = mybir.dt.float16
I16 = mybir.dt.int16

# z-scale engine per chunk parity: 'v' = DVE, 'g' = GpSimd
Z_ENG = "v"


def build_kernel():
    nc = bacc.Bacc("TRN2", num_swdge_queues=4)

    xb0 = nc.dram_tensor("xb0", [BANK0, DIN], F16, kind="ExternalInput")
    xb1 = nc.dram_tensor("xb1", [BANK1, DIN], F16, kind="ExternalInput")
    xtl = nc.dram_tensor("xtl", [DIN, NLOC], F16, kind="ExternalInput")
    wh = nc.dram_tensor("wh", [2, 2, 128, HC], F16, kind="ExternalInput")
    wes = nc.dram_tensor("wes", [2, 2, 128, H], F16, kind="ExternalInput")
    wed = nc.dram_tensor("wed", [2, 128, 2 * H], F16, kind="ExternalInput")
    bias = nc.dram_tensor("bias", [128, HC], F32, kind="ExternalInput")
    gidx = nc.dram_tensor("gidx", [2, NBIN, 128, 80], I16, kind="ExternalInput")
    mks = nc.dram_tensor("mks", [2, NBIN, 128, CPB, 128], F16,
                         kind="ExternalInput")
    mkt = nc.dram_tensor("mkt", [2, NBIN, 128, CPB, 128], F16,
                         kind="ExternalInput")
    out = nc.dram_tensor("out", [NLOC, HC], F32, kind="ExternalOutput")

    with tile.TileContext(nc) as tc:
        with tc.tile_pool(name="const", bufs=1) as cp:
            nc.gpsimd.load_library(library_config.mlp)

            # weights
            wh_sb = [cp.tile([128, 2, HC], F16, tag=f"wh{d}", name=f"wh{d}")
                     for d in range(2)]
            wes_sb = [cp.tile([128, 2, H], F16, tag=f"wes{d}", name=f"wes{d}")
                      for d in range(2)]
            for d in range(2):
                for k in range(2):
                    nc.sync.dma_start(wh_sb[d][:, k, :], wh[d, k, :, :])
                    nc.sync.dma_start(wes_sb[d][:, k, :], wes[d, k, :, :])
            wed_sb = cp.tile([128, 2, 2 * H], F16)
            for k in range(2):
                nc.sync.dma_start(wed_sb[:, k, :], wed[k, :, :])
            bias_sb = cp.tile([128, HC], F32)
            nc.sync.dma_start(bias_sb[:], bias[:])

            # ---------------- Phase A-lite: ed for local (permuted) nodes ---
            ed_sb = cp.tile([128, NBIN, 2 * H], F16, name="ed_sb")
            with (
                tc.tile_pool(name="pA", bufs=3) as pa,
                tc.tile_pool(name="psA", bufs=2, space="PSUM") as psa,
            ):
                for t in range(NBIN):
                    xlt = pa.tile([128, 2, 128], F16, tag="xlt")
                    for k in range(2):
                        nc.sync.dma_start(
                            xlt[:, k, :],
                            xtl[k * 128:(k + 1) * 128, t * 128:(t + 1) * 128])
                    ped = psa.tile([128, 2 * H], F32, tag="ped")
                    for k in range(2):
                        nc.tensor.matmul(ped[:], xlt[:, k, :], wed_sb[:, k, :],
                                         start=(k == 0), stop=(k == 1))
                    nc.vector.tensor_copy(ed_sb[:, t, :], ped[:])

            # ---------------- Phase B ----------------
            with (
                tc.tile_pool(name="pBg", bufs=3) as pg,
                tc.tile_pool(name="pBm", bufs=3) as pm,
                tc.tile_pool(name="pBs", bufs=4) as psb,
                tc.tile_pool(name="pBz", bufs=CPB + 1) as pz,
                tc.tile_pool(name="pBo", bufs=3) as po,
                tc.tile_pool(name="psH", bufs=4, space="PSUM") as psh,
                tc.tile_pool(name="psG", bufs=2, space="PSUM") as psg,
                tc.tile_pool(name="psL", bufs=2, space="PSUM") as psl,
            ):
                for b in range(NBIN):
                    stage = [None, None]
                    for d in range(2):
                        gi = pm.tile([128, 80], I16, tag="gi")
                        nc.sync.dma_start(gi[:], gidx[d, b, :, :])
                        xg = [pg.tile([128, 2, CB * 128], F16, tag=f"xg{bk}",
                                      name=f"xg{bk}") for bk in range(2)]
                        qq = (2 * b + d) * 2
                        nc.gpsimd.dma_gather(
                            xg[0][:], xb0[:], gi[:, 0:40], CB * 128, CB * 128,
                            DIN, transpose=True, queue_num=qq % 4)
                        nc.gpsimd.dma_gather(
                            xg[1][:], xb1[:], gi[:, 40:80], CB * 128, CB * 128,
                            DIN, transpose=True, queue_num=(qq + 1) % 4)
                        mks_t = pm.tile([128, CPB, 128], F16, tag="mks")
                        nc.sync.dma_start(mks_t[:], mks[d, b, :, :, :])
                        mkt_t = pm.tile([128, CPB, 128], F16, tag="mkt")
                        nc.sync.dma_start(mkt_t[:], mkt[d, b, :, :, :])

                        # PSUM banks (never touch a bank while an accumulation
                        # group is open in it):
                        #   num bank: [0:260] = M^T @ [z | p], one group/dir
                        #   lgp tile: es+ed per chunk, own bank via pool ring
                        num = psg.tile([128, 512], F32, tag="num")

                        zs = []
                        for cc in range(CPB):
                            bk, c0 = divmod(cc, CB)
                            sl = slice(c0 * 128, (c0 + 1) * 128)
                            # lgp = ed (via mask transpose) + es, one region
                            lgp = psl.tile([128, H], F32, tag="lgp")
                            nc.tensor.matmul(
                                lgp[:], mkt_t[:, cc, :],
                                ed_sb[:, b, H * d:H * d + H],
                                start=True, stop=False)
                            if cc % 2 == 0:
                                hpair = psh.tile([128, 2, HC], F32, tag="hp")
                            hp = hpair[:, cc % 2, :]
                            for k in range(2):
                                nc.tensor.matmul(
                                    hp, xg[bk][:, k, sl], wh_sb[d][:, k, :],
                                    start=(k == 0), stop=(k == 1))
                                nc.tensor.matmul(
                                    lgp[:], xg[bk][:, k, sl],
                                    wes_sb[d][:, k, :],
                                    start=False, stop=(k == 1))
                            # p = exp(leakyrelu(es+ed)) on Scalar
                            lr = psb.tile([128, H], F16, tag="lr")
                            nc.scalar.activation(
                                lr[:], lgp[:],
                                mybir.ActivationFunctionType.Prelu,
                                alpha=SLOPE)
                            z = pz.tile([128, HC + H], F16, tag="z")
                            zs.append(z)
                            nc.scalar.activation(
                                z[:, HC:HC + H], lr[:],
                                mybir.ActivationFunctionType.Exp)
                            nc.vector.tensor_tensor(
                                z[:, 0:HC].rearrange("p (h c) -> p h c", h=H),
                                hp.rearrange("p (h c) -> p h c", h=H),
                                z[:, HC:HC + H].unsqueeze(2).broadcast_to(
                                    [128, H, C]),
                                mybir.AluOpType.mult)
                        # aggregation matmuls last: by the time these issue,
                        # the z chains have drained (software pipelining)
                        for cc in range(CPB):
                            nc.tensor.matmul(num[:, 0:HC + H],
                                             mks_t[:, cc, :], zs[cc][:],
                                             start=(cc == 0),
                                             stop=(cc == CPB - 1))
                        # normalize: stage = num / den  (den>0: self-loop)
                        den2 = po.tile([128, H], F32, tag="den2")
                        nc.vector.tensor_scalar(
                            out=den2[:], in0=num[:, HC:HC + H], scalar1=2.0,
                            scalar2=1e-12, op0=mybir.AluOpType.mult,
                            op1=mybir.AluOpType.max)
                        rec = po.tile([128, H], F32, tag="rec")
                        nc.vector.reciprocal(rec[:], den2[:])
                        stage[d] = po.tile([128, HC], F32, tag=f"st{d}",
                                           name=f"st{d}")
                        nc.vector.tensor_tensor(
                            stage[d][:].rearrange("p (h c) -> p h c", h=H),
                            num[:, 0:HC].rearrange("p (h c) -> p h c", h=H),
                            rec[:].unsqueeze(2).broadcast_to([128, H, C]),
                            mybir.AluOpType.mult)

                    ot = po.tile([128, HC], F32, tag="ot")
                    nc.gpsimd.tensor_tensor(ot[:], stage[0][:], stage[1][:],
                                            mybir.AluOpType.add)
                    ot2 = po.tile([128, HC], F32, tag="ot2")
                    nc.gpsimd.tensor_tensor(ot2[:], ot[:], bias_sb[:],
                                            mybir.AluOpType.add)
                    nc.sync.dma_start(out[b * 128:(b + 1) * 128, :], ot2[:])

    nc.compile()
    return nc


# ---------------------------------------------------------------- host side

def _wrap16(arr):
    """int idx array [n] -> dma_gather layout [128, n/16] int16 (replicated)."""
    n = len(arr)
    m = arr.reshape(n // 16, 16).astype(np.int16).T  # [16, n/16]
    return np.tile(m, (8, 1))


def prep_inputs(x, edge_index, W1, a_src1, a_dst1, b1, W2, a_src2, a_dst2, b2):
    x = np.asarray(x, np.float32)
    ei = np.asarray(edge_index)
    src, dst = ei[0].astype(np.int64), ei[1].astype(np.int64)
    loops = np.arange(N, dtype=np.int64)
    dirs = [
        (np.concatenate([src, loops]), np.concatenate([dst, loops])),
        (np.concatenate([dst, loops]), np.concatenate([src, loops])),
    ]

    x16 = x.astype(np.float16)
    xpad = np.zeros((BANK0 + BANK1, DIN), np.float16)
    xpad[:N] = x16
    xb0_h, xb1_h = xpad[:BANK0], xpad[BANK0:]

    Ws = [np.asarray(W1, np.float32), np.asarray(W2, np.float32)]
    asrc = [np.asarray(a_src1, np.float32), np.asarray(a_src2, np.float32)]
    adst = [np.asarray(a_dst1, np.float32), np.asarray(a_dst2, np.float32)]
    wh_h = np.zeros((2, 2, 128, HC), np.float16)
    wes_h = np.zeros((2, 2, 128, H), np.float16)
    wed_h = np.zeros((2, 128, 2 * H), np.float16)
    for d in range(2):
        Wd = Ws[d]
        w_es = np.stack([Wd[:, h * C:(h + 1) * C] @ asrc[d][h]
                         for h in range(H)], 1)       # [DIN, H]
        w_ed = np.stack([Wd[:, h * C:(h + 1) * C] @ adst[d][h]
                         for h in range(H)], 1)
        for k in range(2):
            wh_h[d, k] = Wd[k * 128:(k + 1) * 128, :].astype(np.float16)
            wes_h[d, k] = w_es[k * 128:(k + 1) * 128, :].astype(np.float16)
            wed_h[k, :, H * d:H * d + H] = \
                w_ed[k * 128:(k + 1) * 128, :].astype(np.float16)
    bias_h = np.broadcast_to(
        (0.5 * (np.asarray(b1) + np.asarray(b2))).astype(np.float32),
        (128, HC)).copy()

    # per-node degree by (dir, src-bank)
    deg = np.zeros((N, 4), np.int64)
    for j, (ss, dd) in enumerate(dirs):
        for bk in range(2):
            m = (ss >= BANK0) == (bk == 1)
            deg[:, 2 * j + bk] = np.bincount(dd[m], minlength=N)

    in_maps, perms = [], []
    for core in range(NCORES):
        lo = core * NPC
        nodes = np.arange(lo, lo + NPC)
        order = nodes[np.argsort(-deg[nodes].sum(1), kind="stable")]
        degs = deg[order]
        bins_load = np.zeros((NBIN, 4), np.int64)
        bins_cnt = np.zeros(NBIN, np.int64)
        node_blk = np.full(N, -1, np.int64)
        node_slot = np.full(N, -1, np.int64)
        for i_n in range(len(order)):
            dgl = degs[i_n]
            ok = (bins_cnt < 128) & ((bins_load + dgl) <= CB * 128).all(1)
            assert ok.any(), "bin packing failed; raise NBIN/CB"
            cand = np.where(ok)[0]
            nl = (bins_load[cand] + dgl).max(1) * 1000 + bins_cnt[cand]
            i = cand[np.argmin(nl)]
            node_blk[order[i_n]] = i
            node_slot[order[i_n]] = bins_cnt[i]
            bins_load[i] += dgl
            bins_cnt[i] += 1

        perm = np.full(NBIN * 128, -1, np.int64)
        perm[node_blk[nodes] * 128 + node_slot[nodes]] = nodes
        perms.append(perm)

        xtl_h = np.zeros((DIN, NLOC), np.float16)
        valid = perm >= 0
        xtl_h[:, valid] = x16[perm[valid]].T

        g_idx = np.zeros((2, NBIN, 128, 80), np.int16)
        mks_h = np.zeros((2, NBIN, CPB, 128, 128), np.float16)
        mkt_h = np.zeros((2, NBIN, CPB, 128, 128), np.float16)
        for d, (ss, dd) in enumerate(dirs):
            sel = (dd >= lo) & (dd < lo + NPC)
            es_, ed_ = ss[sel], dd[sel]
            blk = node_blk[ed_]
            bank = (es_ >= BANK0).astype(np.int64)
            eo = np.lexsort((bank, blk))
            es_, ed_, blk, bank = es_[eo], ed_[eo], blk[eo], bank[eo]
            seg = blk * 2 + bank
            segbnd = np.flatnonzero(np.diff(seg, prepend=-1))
            within = np.arange(len(seg)) - np.repeat(segbnd, np.diff(
                np.append(segbnd, len(seg))))
            assert (within < CB * 128).all()
            slot = within + np.where(bank == 0, 0, CB * 128)
            srcrel = np.where(bank == 0, es_, es_ - BANK0)
            s_idx = np.zeros((NBIN, CPB * 128), np.int64)
            s_idx[blk, slot] = srcrel
            # one-hot masks: edge at (chunk cc, lane e) -> dst slot
            cc_all = slot // 128
            lane = slot % 128
            dslot = node_slot[ed_]
            mks_h[d, blk, cc_all, lane, dslot] = 1.0
            mkt_h[d, blk, cc_all, dslot, lane] = 1.0
            for bb in range(NBIN):
                g_idx[d, bb, :, 0:40] = _wrap16(s_idx[bb, 0:CB * 128])
                g_idx[d, bb, :, 40:80] = _wrap16(s_idx[bb, CB * 128:])

        in_maps.append({
            "xb0": xb0_h, "xb1": xb1_h, "xtl": xtl_h,
            "wh": wh_h, "wes": wes_h, "wed": wed_h, "bias": bias_h,
            "gidx": g_idx,
            "mks": np.ascontiguousarray(mks_h.transpose(0, 1, 3, 2, 4)),
            "mkt": np.ascontiguousarray(mkt_h.transpose(0, 1, 3, 2, 4)),
        })
    return in_maps, perms


_NC_CACHE = {}


def kernel(**inputs):
    in_maps, perms = prep_inputs(**inputs)
    key = "k2"
    if key not in _NC_CACHE:
        _NC_CACHE[key] = build_kernel()
    nc = _NC_CACHE[key]
    res = run_bass_kernel_spmd(nc, in_maps, list(range(NCORES)))
    result = np.empty((N, HC), np.float32)
    for core in range(NCORES):
        o = res.results[core]["out"]
        p = perms[core]
        valid = p >= 0
        result[p[valid]] = o[valid]
    return result
